# revision 1
# baseline (speedup 1.0000x reference)
"""RNN-T Joiner kernel for Trainium2, data-parallel over (B, T) on 8 cores.

reference:
    logit = tanh(enc[:, :, None, :] + dec[:, None, :, :])   # (B,T,U,C)
    out   = einsum('btuc,vc->btuv', logit, W) + b           # (B,T,U,V)

Shapes (hardcoded): B=4, T=256, U=64, C=512, V=1024.

Sharding: core k handles b = k//2, t rows [ (k%2)*128, (k%2)*128+128 ).
W / bias replicated. No collectives.

Per-core device kernel (C on partitions for the logit):
  - logitT[c, t] = tanh(encT[c, t] + decT[c, u])  -- scalar engine, fused
    per-partition bias add.
  - out[t, v]: vh-major matmuls into single-bank PSUM tiles so each
    512-wide half evicts on DVE (fused bias add) and stores as soon as its
    4 contraction chunks finish — finer overlap than a full [128,1024]
    eviction, and the kernel tail drains half a tile earlier.
  - inputs bitcast to float32r (full PE rate at out-free-dim >= 256).
  - startup: tanh table preloaded via a dummy act; PE p-state warmed with
    throwaway matmuls; W chunks split across SP+Pool queues.
  - output stores on the SP queue (input loads ride Pool).
"""

import numpy as np

B, T, U, C, V = 4, 256, 64, 512, 1024
NCORES = 8
TS = 128  # t rows per core
CCH = C // 128  # 4 contraction chunks
VH = V // 512  # 2 psum-width chunks

_CACHE = {}


def _build(repeat=1):
    from contextlib import ExitStack

    import concourse.bacc as bacc
    import concourse.mybir as mybir
    import concourse.tile as tile

    dt = mybir.dt
    f32 = dt.float32
    f32r = dt.float32r

    nc = bacc.Bacc("TRN2", target_bir_lowering=False, debug=False, num_devices=NCORES)
    enc_t = nc.declare_dram_parameter("enc_t", [C, TS], f32, isOutput=False)
    dec_t = nc.declare_dram_parameter("dec_t", [C, U], f32, isOutput=False)
    wt = nc.declare_dram_parameter("wt", [C, V], f32r, isOutput=False)
    bias_rep = nc.declare_dram_parameter("bias_rep", [128, V], f32, isOutput=False)
    out = nc.declare_dram_parameter("out", [TS, U, V], f32, isOutput=True)

    with tile.TileContext(nc) as tc, ExitStack() as ctx:
        const = ctx.enter_context(tc.tile_pool(name="const", bufs=1))
        logit_pool = ctx.enter_context(tc.tile_pool(name="logit", bufs=8))
        psum_pool = ctx.enter_context(tc.tile_pool(name="psum", bufs=3, space="PSUM"))
        warm_pool = ctx.enter_context(tc.tile_pool(name="warm", bufs=1, space="PSUM"))
        out_pool = ctx.enter_context(tc.tile_pool(name="out", bufs=12))

        wt_sb = const.tile([128, CCH * V], f32r, tag="wt")
        enc_sb = const.tile([128, CCH * TS], f32, tag="enc")
        dec_sb = const.tile([128, CCH * U], f32, tag="dec")
        bias_sb = const.tile([128, V], f32, tag="bias")
        scratch = const.tile([128, 1], f32, tag="scratch")

        # Preload the tanh activation table while input DMAs run.
        nc.vector.memset(scratch[:], 0.0)
        nc.scalar.activation(
            scratch[:], scratch[:], mybir.ActivationFunctionType.Tanh
        )
        # Warm the PE clock (p-state ramps with continuous work) during the
        # input-DMA window with throwaway matmuls on a spare PSUM bank.
        warm_sb = const.tile([128, 512], f32, tag="warm_sb")
        warm = warm_pool.tile([128, 512], f32, tag="warm")
        nc.vector.memset(warm_sb[:], 0.0)
        for _ in range(5):
            nc.tensor.matmul(
                warm[:],
                lhsT=warm_sb[:, 0:128].bitcast(f32r),
                rhs=warm_sb[:].bitcast(f32r),
                start=True,
                stop=True,
            )

        nc.gpsimd.dma_start(enc_sb[:, 0:TS], enc_t[0:128, :])
        nc.gpsimd.dma_start(
            dec_sb[:].rearrange("p (c u) -> p c u", c=CCH),
            dec_t[:].rearrange("(c p) u -> p c u", p=128),
        )
        nc.gpsimd.dma_start(
            enc_sb[:, TS:].rearrange("p (c t) -> p c t", c=CCH - 1),
            enc_t[128:, :].rearrange("(c p) t -> p c t", p=128),
        )
        nc.sync.dma_start(wt_sb[:, 0:512], wt[0:128, 0:512])
        nc.sync.dma_start(wt_sb[:, 512:V], wt[0:128, 512:V])
        nc.sync.dma_start(wt_sb[:, V : 2 * V], wt[128:256, :])
        for c in range(2, CCH):
            nc.gpsimd.dma_start(
                wt_sb[:, c * V : (c + 1) * V], wt[c * 128 : (c + 1) * 128, :]
            )
        nc.gpsimd.dma_start(bias_sb[:], bias_rep[:])

        for u in [u for _ in range(repeat) for u in range(U)]:
            lg = logit_pool.tile([128, CCH * TS], f32r, tag="lg")
            for c in range(CCH):
                nc.scalar.activation(
                    lg[:, c * TS : (c + 1) * TS],
                    enc_sb[:, c * TS : (c + 1) * TS],
                    mybir.ActivationFunctionType.Tanh,
                    bias=dec_sb[:, c * U + u : c * U + u + 1],
                )
            for vh in range(VH):
                ps = psum_pool.tile([128, 512], f32, tag=f"ps{vh}")
                for c in range(CCH):
                    nc.tensor.matmul(
                        ps[:],
                        lhsT=lg[:, c * TS : (c + 1) * TS],
                        rhs=wt_sb[:, c * V + vh * 512 : c * V + vh * 512 + 512],
                        start=(c == 0),
                        stop=(c == CCH - 1),
                    )
                ob = out_pool.tile([128, 512], f32, tag=f"ob{vh}")
                nc.vector.tensor_add(
                    ob[:], ps[:], bias_sb[:, vh * 512 : (vh + 1) * 512]
                )
                nc.sync.dma_start(out[:, u, vh * 512 : (vh + 1) * 512], ob[:])

    nc.finalize()
    return nc


def _get_nc():
    if "nc" not in _CACHE:
        _CACHE["nc"] = _build()
    return _CACHE["nc"]


def kernel(**inputs):
    enc = np.asarray(inputs["enc_out"], dtype=np.float32)
    dec = np.asarray(inputs["dec_out"], dtype=np.float32)
    W = np.asarray(inputs["W"], dtype=np.float32)
    b = np.asarray(inputs["b"], dtype=np.float32)

    nc = _get_nc()

    wt_np = np.ascontiguousarray(W.T)
    bias_np = np.ascontiguousarray(np.broadcast_to(b, (128, V)))
    in_maps = []
    for k in range(NCORES):
        bb, t0 = k // 2, (k % 2) * TS
        in_maps.append(
            {
                "enc_t": np.ascontiguousarray(enc[bb, t0 : t0 + TS, :].T),
                "dec_t": np.ascontiguousarray(dec[bb].T),
                "wt": wt_np,
                "bias_rep": bias_np,
            }
        )

    from concourse.bass_utils import run_bass_kernel_spmd

    res = run_bass_kernel_spmd(nc, in_maps, list(range(NCORES)))
    _CACHE["last_result"] = res

    out = np.empty((B, T, U, V), np.float32)
    for k in range(NCORES):
        bb, t0 = k // 2, (k % 2) * TS
        out[bb, t0 : t0 + TS] = res.results[k]["out"]
    return out

